# revision 67
# baseline (speedup 1.0000x reference)
"""NeuronSAT GNN message passing on 8 Trainium2 NeuronCores.

Sharding: data-parallel over graphs — graph g lives entirely on core g.
All state (h, c), weights, and the per-graph bipartite incidence matrices
are SBUF-resident for all 26 rounds; HBM traffic is a one-time load per
core plus a 4-byte result store.

Layout: feature-major [128=D, nodes]. Literal<->clause aggregations are
dense matmuls against the (zero-padded) incidence matrix A [800,1200]
(and its transpose). The incidence matrices and node-major message tiles
are fp8e4 (exact for the 0..5 edge multiplicities; messages quantized at
PSUM evacuation), which lets the aggregation accumulation chains run in
DoubleRow mode: two 128-row contraction tiles per pass, halving PE time
for the dominant matmuls. The message-MLP last-layer bias is folded
analytically (every clause has exactly K=5 edges; literal degrees give a
precomputed outer-product bias).

The round loop is rotated: the literal message MLP for round r+1 is
emitted at the end of round r's body, so its matmuls overlap the literal
LSTM's elementwise tail (ACT sigmoids + DVE cell update) instead of the
PE idling there. Keep-warm matmuls cover the remaining gaps so the PE
HAM clock gate stays at 2.4GHz.

Matmul accumulation is fp32 PSUM; biases stay fp32; LSTM elementwise is
bf16 (2x DVE mode).
"""

import sys

sys.path.insert(0, "/opt/trn_rl_repo")

import ml_dtypes
import numpy as np

import concourse.bacc as bacc
import concourse.mybir as mybir
import concourse.tile as tile
from concourse.tile import add_dep_helper
from concourse.bass_utils import run_bass_kernel_spmd

# Problem dims (fixed by the reference).
NG = 8          # graphs == cores
NV = 400        # vars per graph
NCL = 1200      # clauses per graph
KLIT = 5        # literals per clause
NLIT = 2 * NV   # 800 literal nodes per graph
NNG = NLIT + NCL  # 2000 nodes per graph
D = 128
ROUNDS = 26
LCH = (NLIT + 127) // 128   # 7 literal 128-chunks (last has 32)
CCH = (NCL + 127) // 128    # 10 clause 128-chunks (last has 48)

F32 = mybir.dt.float32
BF16 = mybir.dt.bfloat16
FP8 = mybir.dt.float8e4
MM_DT = BF16   # vote-head matmul operand dtype
MSG_DT = FP8   # node-major messages + incidence (DoubleRow aggregation)
ACT_DT = FP8   # recurrent activations (h, x1/x2, agg) — DoubleRow gates
EW_DT = BF16   # LSTM gates / cell state dtype
AF = mybir.ActivationFunctionType
ALU = mybir.AluOpType
DR = mybir.MatmulPerfMode.DoubleRow

# Clause columns: 512-aligned chunks, compact in psum.
CL_CH = [(0, 512), (512, 512), (1024, 176)]
# Literal columns: 400-wide (flip partner of [0:400] is [400:800]).
LIT_CH = [(0, 400), (400, 400)]

# keep-warm tuning knobs (matmuls of 128 cols each)
W_LAYER = 1
W_GATES = 2
W_AGG = 1
W_NM = 1
W_CELEM = 6
W_POSTCL = 9
W_LELEM = 12


def _np_dt(dt):
    if dt == BF16:
        return ml_dtypes.bfloat16
    if dt == FP8:
        return ml_dtypes.float8_e4m3
    return np.float32


def build_nc(rounds=ROUNDS, debug_state=False):
    nc = bacc.Bacc(None, target_bir_lowering=False)

    def din(name, shape, dt=MM_DT):
        return nc.declare_dram_parameter(name, list(shape), dt, isOutput=False)

    a_lc_d = din("a_lc", [128, LCH, NCL], MSG_DT)
    a_cl_d = din("a_cl", [128, CCH, NLIT], MSG_DT)
    h0_lit_d = din("h0_lit", [128, NLIT], ACT_DT)
    h0_cl_d = din("h0_cl", [128, NCL], ACT_DT)
    lmsgT_d = din("lmsgT", [128, 3, 128], ACT_DT)
    lmsg_b_d = din("lmsg_b", [128, 2], F32)
    cmsgT_d = din("cmsgT", [128, 3, 128], ACT_DT)
    cmsg_b_d = din("cmsg_b", [128, 2], F32)
    aggc_b_d = din("aggc_b", [128, 1], F32)
    aggl_b_d = din("aggl_b", [128, NLIT], F32)
    # clause gate weights as (wih, whh) DoubleRow pairs
    cu_pair_d = din("cu_pair", [128, 4, 2, 128], ACT_DT)
    cu_b_d = din("cu_b", [128, 4], F32)
    # literal gate weights: (whh, wihb) pair for chunk 0, (wihb, whh) for
    # chunk 1, plus the regular wiha term on agg_l
    lu_wihTa_d = din("lu_wihTa", [128, 4, 128], ACT_DT)
    lu_pair0_d = din("lu_pair0", [128, 4, 2, 128], ACT_DT)
    lu_pair1_d = din("lu_pair1", [128, 4, 2, 128], ACT_DT)
    lu_b_d = din("lu_b", [128, 4], F32)
    warm_d = din("warmsrc", [128, 2, 128], BF16)
    vw0T_d = din("vw0T", [128, 128])
    vw1T_d = din("vw1T", [128, 128])
    vw2T_d = din("vw2T", [128, 1])
    vb_d = din("vb", [128, 2], F32)

    out_d = nc.declare_dram_parameter("out", [1, 1], F32, isOutput=True)
    if debug_state:
        dbg = {
            n: nc.declare_dram_parameter(n, [128, w], F32, isOutput=True)
            for n, w in (("dbg_hl", NLIT), ("dbg_hc", NCL),
                         ("dbg_cl", NLIT), ("dbg_cc", NCL))
        }

    with tile.TileContext(nc) as tc:
        with tc.tile_pool(name="singles", bufs=1) as singles, \
             tc.tile_pool(name="work", bufs=3) as work, \
             tc.tile_pool(name="ps", bufs=2, space="PSUM") as psp:

            def load(name, shape, dram, dt=MM_DT):
                t = singles.tile(list(shape), dt, tag=name, name=name)
                nc.sync.dma_start(out=t[:], in_=dram[:])
                return t

            lmsgT = load("lmsgT", [128, 3, 128], lmsgT_d, ACT_DT)
            lmsg_b = load("lmsg_b", [128, 2], lmsg_b_d, F32)
            cmsgT = load("cmsgT", [128, 3, 128], cmsgT_d, ACT_DT)
            cmsg_b = load("cmsg_b", [128, 2], cmsg_b_d, F32)
            aggc_b = load("aggc_b", [128, 1], aggc_b_d, F32)
            aggl_b = load("aggl_b", [128, NLIT], aggl_b_d, F32)
            cu_pair = load("cu_pair", [128, 4, 2, 128], cu_pair_d, ACT_DT)
            cu_b = load("cu_b", [128, 4], cu_b_d, F32)
            lu_wihTa = load("lu_wihTa", [128, 4, 128], lu_wihTa_d, ACT_DT)
            lu_pair0 = load("lu_pair0", [128, 4, 2, 128], lu_pair0_d, ACT_DT)
            lu_pair1 = load("lu_pair1", [128, 4, 2, 128], lu_pair1_d, ACT_DT)
            lu_b = load("lu_b", [128, 4], lu_b_d, F32)
            vw0T = load("vw0T", [128, 128], vw0T_d)
            vw1T = load("vw1T", [128, 128], vw1T_d)
            vw2T = load("vw2T", [128, 1], vw2T_d)
            vb = load("vb", [128, 2], vb_d, F32)
            warmsrc = load("warmsrc", [128, 2, 128], warm_d, BF16)
            # big incidence matrices last: the prologue only needs the
            # small weights + h0, so their DMAs shouldn't queue behind 2MB
            # (the startup warm burst below runs during these DMA waits so
            # the prologue starts at the boosted PE clock)
            a_lc = load("a_lc", [128, LCH, NCL], a_lc_d, MSG_DT)
            a_cl = load("a_cl", [128, CCH, NLIT], a_cl_d, MSG_DT)

            # xh: interleaved [agg_c ; h_cl] fp8 — one DoubleRow stream
            # feeds both clause-gate terms in a single matmul pass
            xh = singles.tile([128, 2, NCL], ACT_DT, tag="xh", name="xh")
            h_cl = xh[:, 1, :]
            h_lit = load("h_lit", [128, NLIT], h0_lit_d, ACT_DT)
            nc.sync.dma_start(out=h_cl, in_=h0_cl_d[:])
            c_lit = singles.tile([128, NLIT], EW_DT, tag="c_lit", name="c_lit")
            c_cl = singles.tile([128, NCL], EW_DT, tag="c_cl", name="c_cl")
            nc.vector.memset(c_lit[:], 0.0)
            nc.vector.memset(c_cl[:], 0.0)

            def ps_g(w=512):
                return psp.tile([128, 512], F32, tag="pg", name="pg", bufs=3)

            def ps_m(w=512):
                return psp.tile([128, 512], F32, tag="pm", name="pm", bufs=4)

            pw = psp.tile([128, 128], F32, tag="pW", name="pW", bufs=1)

            last_mm = [None]

            def mm(*args, **kw):
                inst = nc.tensor.matmul(*args, **kw)
                last_mm[0] = inst
                return inst

            def pe_warm(n):
                """Keep-warm matmuls: harmless PE work that keeps the HAM
                activity window busy so the clock gate stays at 2.4GHz.
                An ordering-only edge to the latest real matmul pins the
                burst at this program position."""
                for k in range(n):
                    d = nc.tensor.matmul(pw[:], warmsrc[:, 0, :],
                                         warmsrc[:, 1, :],
                                         start=True, stop=True)
                    if k == 0 and last_mm[0] is not None:
                        add_dep_helper(d.ins, last_mm[0].ins, sync=False,
                                       reason="pin keep-warm burst")

            def warm_after(dep, n):
                """Warm matmuls gated (semaphore) on a DVE/ACT instruction:
                spreads PE activity through an elementwise-dominated phase
                so the HAM MID window never sees a fully idle PE."""
                for k in range(n):
                    d = nc.tensor.matmul(pw[:], warmsrc[:, 0, :],
                                         warmsrc[:, 1, :],
                                         start=True, stop=True)
                    if k == 0 and dep is not None:
                        add_dep_helper(d.ins, dep.ins, sync=True,
                                       reason="spread keep-warm")

            def layer_chunk(dst, src_ap, wT_l, bias, c0, w, act=False):
                """One MLP layer on one column chunk: matmul + fused
                bias+relu (on ScalarE when `act`, else DVE)."""
                ps = ps_m()
                mm(ps[:, :w], wT_l, src_ap,
                   start=True, stop=True)
                if act:
                    nc.scalar.activation(dst[:, c0:c0 + w], ps[:, :w],
                                         AF.Relu, bias=bias)
                else:
                    nc.vector.tensor_scalar(dst[:, c0:c0 + w], ps[:, :w],
                                            bias, 0.0,
                                            op0=ALU.add, op1=ALU.max)
                pe_warm(W_LAYER)

            def nm_group(m_nm, x2, wT2, ncols, g0, gn):
                """Node-major last-layer chunks g0..g0+gn packed into one
                psum bank, one DVE copy out (cast to fp8)."""
                ps = ps_m()
                for i in range(g0, g0 + gn):
                    k = min(128, ncols - 128 * i)
                    mm(ps[:k, 128 * (i - g0):128 * (i - g0) + 128],
                       x2[:, 128 * i:128 * i + k],
                       wT2, start=True, stop=True)
                ps3 = ps[:].rearrange("p (b c) -> p b c", c=128)
                nc.vector.tensor_copy(m_nm[:, g0:g0 + gn, :], ps3[:, 0:gn, :])
                pe_warm(W_NM)

            def agg_chunk_dr(dst_post, m_t, a_t, nch, nrows, c0, w,
                             bias_mm=None):
                """Aggregation chunk: DoubleRow accumulation chain over
                fp8 message/incidence k-tile pairs + post-evac. An odd
                tail chunk (partial partitions) runs as a regular matmul."""
                ps = ps_m()
                npairs = nch // 2
                has_tail = (nch % 2) == 1
                for p in range(npairs):
                    mm(ps[:, :w], m_t[:, 2 * p:2 * p + 2, :],
                       a_t[:, 2 * p:2 * p + 2, c0:c0 + w],
                       start=(p == 0), stop=(not has_tail and bias_mm is None
                                             and p == npairs - 1),
                       perf_mode=DR)
                if has_tail:
                    p = nch - 1
                    k = min(128, nrows - 128 * p)
                    mm(ps[:, :w], m_t[:k, p, :], a_t[:k, p, c0:c0 + w],
                       start=(npairs == 0), stop=(bias_mm is None))
                if bias_mm is not None:
                    blhsT, brhs = bias_mm
                    mm(ps[:, :w], blhsT, brhs, start=False, stop=True)
                dst_post(ps[:, :w])
                pe_warm(W_AGG)

            def gates_chunk(gset, terms_fn, bias_t, c0, w, warm=W_GATES,
                            spread=0):
                """All four LSTM gates for one column chunk (f,i,g,o).
                Terms are (lhsT, rhs, perf_mode) triples; DoubleRow terms
                cover two 128-row contraction tiles per pass."""
                for gi in (1, 0, 2, 3):
                    ps = ps_g()
                    terms = terms_fn(gi)
                    for t_i, (lhsT, rhs, pm) in enumerate(terms):
                        mm(ps[:, :w], lhsT, rhs,
                           start=(t_i == 0),
                           stop=(t_i == len(terms) - 1),
                           perf_mode=pm)
                    fn = AF.Tanh if gi == 2 else AF.Sigmoid
                    da = nc.scalar.activation(gset[gi][:, c0:c0 + w],
                                              ps[:, :w],
                                              fn, bias=bias_t[:, gi:gi + 1])
                    if spread:
                        warm_after(da, spread)
                pe_warm(warm)

            def gates_chunk_mm(terms_fn, w):
                """Emit just the gate matmuls for one chunk; ACTs follow
                later via gates_chunk_act (splitting lets another chunk's
                elementwise tail run ahead of this chunk's activations)."""
                pss = {}
                for gi in (1, 0, 2, 3):
                    ps = ps_g()
                    terms = terms_fn(gi)
                    for t_i, (lhsT, rhs, pm_) in enumerate(terms):
                        mm(ps[:, :w], lhsT, rhs,
                           start=(t_i == 0),
                           stop=(t_i == len(terms) - 1),
                           perf_mode=pm_)
                    pss[gi] = ps
                return pss

            def gates_chunk_act(gset, pss, bias_t, c0, w, warm=0):
                for gi in (1, 0, 2, 3):
                    fn = AF.Tanh if gi == 2 else AF.Sigmoid
                    nc.scalar.activation(gset[gi][:, c0:c0 + w],
                                         pss[gi][:, :512][:, :w],
                                         fn, bias=bias_t[:, gi:gi + 1])
                pe_warm(warm)

            def elem_chunk(gates, c_t, h_dst, c0, w, warm=6, spread=0):
                """LSTM cell update for one chunk. h_dst is the
                already-sliced destination AP for the new hidden state."""
                i_ = gates[0][:, c0:c0 + w]
                f_ = gates[1][:, c0:c0 + w]
                g_ = gates[2][:, c0:c0 + w]
                o_ = gates[3][:, c0:c0 + w]
                cs = c_t[:, c0:c0 + w]
                t1 = work.tile([128, 512], EW_DT, tag="t1", name="t1")
                t2 = work.tile([128, 512], EW_DT, tag="t2", name="t2")
                d1 = nc.vector.tensor_mul(t1[:, :w], f_, cs)
                d2 = nc.vector.tensor_mul(t2[:, :w], i_, g_)
                d3 = nc.vector.tensor_add(cs, t1[:, :w], t2[:, :w])
                tc2 = work.tile([128, 512], EW_DT, tag="tc2", name="tc2")
                d4 = nc.scalar.activation(tc2[:, :w], cs, AF.Tanh)
                d5 = nc.vector.tensor_mul(h_dst, o_, tc2[:, :w])
                if spread:
                    for d in (d1, d2, d3, d4, d5):
                        warm_after(d, spread)
                pe_warm(warm)

            # clause-side node-major groups align with CL_CH chunks
            NM_CL = [(0, 4), (4, 4), (8, 2)]
            NM_LIT = [(0, 4), (4, 3)]

            # persistent message tiles (produced at end of round r, consumed
            # by round r+1's aggregation)
            m_nm = work.tile([128, LCH, 128], MSG_DT, tag="m_nmL",
                             name="m_nmL", bufs=1)
            m2_nm = work.tile([128, CCH, 128], MSG_DT, tag="m_nmC",
                              name="m_nmC", bufs=1)

            def lit_mlp():
                """Literal message MLP: h_lit -> m_nm (node-major fp8)."""
                x1 = work.tile([128, NLIT], ACT_DT, tag="mx1", name="mx1",
                               bufs=1)
                x2 = work.tile([128, NLIT], ACT_DT, tag="mx2", name="mx2",
                               bufs=1)
                for (c0, w) in LIT_CH:
                    layer_chunk(x1, h_lit[:, c0:c0 + w], lmsgT[:, 0, :],
                                lmsg_b[:, 0:1], c0, w, act=True)
                for (c0, w) in LIT_CH:
                    layer_chunk(x2, x1[:, c0:c0 + w], lmsgT[:, 1, :],
                                lmsg_b[:, 1:2], c0, w, act=True)
                for (g0, gn) in NM_LIT:
                    nm_group(m_nm, x2, lmsgT[:, 2, :], NLIT, g0, gn)

            lit_mlp()  # prologue: round 0's messages

            for r in range(rounds):
                # ---- agg into clauses (DoubleRow chains) into xh slot 0 ----
                def aggc_post(psl, c0, w, split):
                    if split:
                        # chain-critical first chunk: evacuate both halves
                        # in parallel on DVE + ACT
                        h = (w // 2 + 3) & ~3
                        nc.vector.tensor_scalar(
                            xh[:, 0, c0:c0 + h], psl[:, :h], aggc_b[:, 0:1],
                            None, op0=ALU.add)
                        nc.scalar.activation(
                            xh[:, 0, c0 + h:c0 + w], psl[:, h:w],
                            AF.Identity, bias=aggc_b[:, 0:1])
                    else:
                        nc.vector.tensor_scalar(
                            xh[:, 0, c0:c0 + w], psl, aggc_b[:, 0:1], None,
                            op0=ALU.add)

                for ci, (c0, w) in enumerate(CL_CH):
                    agg_chunk_dr(
                        (lambda psl, c0=c0, w=w, ci=ci: aggc_post(
                            psl, c0, w, ci == 0)),
                        m_nm, a_lc, LCH, NLIT, c0, w)

                # ---- clause LSTM + C_msg MLP, chunk-major interleaved ----
                cgates = {gi: work.tile([128, NCL], EW_DT, tag=f"cg{gi}",
                                        name=f"cg{gi}", bufs=1)
                          for gi in range(4)}
                y1 = work.tile([128, NCL], ACT_DT, tag="my1", name="my1",
                               bufs=1)
                y2 = work.tile([128, NCL], ACT_DT, tag="my2", name="my2",
                               bufs=1)

                def cgate(ci):
                    c0, w = CL_CH[ci]
                    gates_chunk(
                        cgates,
                        lambda gi: [
                            (cu_pair[:, gi, :, :], xh[:, :, c0:c0 + w], DR),
                        ], cu_b, c0, w)

                def celem(ci):
                    c0, w = CL_CH[ci]
                    elem_chunk(cgates, c_cl, xh[:, 1, c0:c0 + w], c0, w,
                               warm=W_CELEM)

                def cL1(ci):
                    c0, w = CL_CH[ci]
                    layer_chunk(y1, xh[:, 1, c0:c0 + w], cmsgT[:, 0, :],
                                cmsg_b[:, 0:1], c0, w)

                def cL2(ci):
                    c0, w = CL_CH[ci]
                    layer_chunk(y2, y1[:, c0:c0 + w], cmsgT[:, 1, :],
                                cmsg_b[:, 1:2], c0, w)

                def cNM(ci):
                    g0, gn = NM_CL[ci]
                    nm_group(m2_nm, y2, cmsgT[:, 2, :], NCL, g0, gn)

                cgate(0); celem(0); cL1(0); cgate(1); celem(1); cL1(1)
                cL2(0); cgate(2); celem(2); cL1(2); cL2(1); cNM(0); cL2(2)
                cNM(1); cNM(2)
                pe_warm(W_POSTCL)

                # ---- agg into literals (DoubleRow chains) ----
                agg_l = work.tile([128, NLIT], ACT_DT, tag="agg_l",
                                  name="agg_l", bufs=1)
                for (c0, w) in LIT_CH:
                    agg_chunk_dr(
                        (lambda psl, c0=c0, w=w: nc.vector.tensor_add(
                            agg_l[:, c0:c0 + w], psl, aggl_b[:, c0:c0 + w])),
                        m2_nm, a_cl, CCH, NCL, c0, w)

                # ---- literal LSTM (gate matmuls all read old h_lit and
                # precede every h_lit write). The (h, h_flip) pair is one
                # DoubleRow stream — same rhs for both chunks, weights
                # swapped host-side.
                lgates = {gi: work.tile([128, NLIT], EW_DT, tag=f"lg{gi}",
                                        name=f"lg{gi}", bufs=1)
                          for gi in range(4)}
                h2 = h_lit[:].rearrange("p (two w) -> p two w", two=2)

                def lterms(ci):
                    c0, w = LIT_CH[ci]
                    lu_pair = lu_pair0 if ci == 0 else lu_pair1
                    return lambda gi: [
                        (lu_pair[:, gi, :, :], h2, DR),
                        (lu_wihTa[:, gi, :], agg_l[:, c0:c0 + w], None),
                    ]

                for ci, (c0, w) in enumerate(LIT_CH):
                    gates_chunk(lgates, lterms(ci), lu_b, c0, w,
                                warm=W_GATES)
                # literal elem interleaved with next round's literal MLP
                # (or, on the last round, with the vote head below)
                last = (r == rounds - 1)
                x1 = work.tile([128, NLIT], ACT_DT, tag="mx1", name="mx1",
                               bufs=1)
                x2 = work.tile([128, NLIT], ACT_DT, tag="mx2", name="mx2",
                               bufs=1)
                for ci, (c0, w) in enumerate(LIT_CH):
                    elem_chunk(lgates, c_lit, h_lit[:, c0:c0 + w], c0, w,
                               warm=W_LELEM)
                    if not last:
                        layer_chunk(x1, h_lit[:, c0:c0 + w], lmsgT[:, 0, :],
                                    lmsg_b[:, 0:1], c0, w, act=True)
                if not last:
                    for (c0, w) in LIT_CH:
                        layer_chunk(x2, x1[:, c0:c0 + w], lmsgT[:, 1, :],
                                    lmsg_b[:, 1:2], c0, w, act=True)
                    for (g0, gn) in NM_LIT:
                        nm_group(m_nm, x2, lmsgT[:, 2, :], NLIT, g0, gn)

            # ---- vote head: mean over literals (sum on device) ----
            hv = work.tile([128, NLIT], MM_DT, tag="hv", name="hv", bufs=1)
            for (c0, w) in LIT_CH:
                nc.vector.tensor_copy(hv[:, c0:c0 + w], h_lit[:, c0:c0 + w])
            v1 = work.tile([128, NLIT], MM_DT, tag="v1", name="v1", bufs=1)
            v2 = work.tile([128, NLIT], MM_DT, tag="v2", name="v2", bufs=1)
            for (wt, src, dst, bi) in ((vw0T, hv, v1, 0), (vw1T, v1, v2, 1)):
                for (c0, w) in LIT_CH:
                    ps = ps_m()
                    nc.tensor.matmul(ps[:, :w], wt[:], src[:, c0:c0 + w],
                                     start=True, stop=True)
                    nc.scalar.activation(dst[:, c0:c0 + w], ps[:, :w],
                                         AF.Relu, bias=vb[:, bi:bi + 1])
            acc = work.tile([1, 2], F32, tag="acc", name="acc", bufs=1)
            for ci, (c0, w) in enumerate(LIT_CH):
                ps = ps_m()
                nc.tensor.matmul(ps[0:1, :w], vw2T[:], v2[:, c0:c0 + w],
                                 start=True, stop=True)
                nc.vector.reduce_sum(acc[:, ci:ci + 1], ps[0:1, :w],
                                     axis=mybir.AxisListType.X)
            total = work.tile([1, 1], F32, tag="total", name="total", bufs=1)
            nc.vector.tensor_add(total[:], acc[:, 0:1], acc[:, 1:2])
            nc.sync.dma_start(out=out_d[:], in_=total[:])

            if debug_state:
                for name, t in (("dbg_hl", h_lit), ("dbg_hc", h_cl),
                                ("dbg_cl", c_lit), ("dbg_cc", c_cl)):
                    w = t.shape[-1]
                    cp = work.tile([128, w], F32, tag="dbg",
                                   name="dbg" + name, bufs=1)
                    nc.vector.tensor_copy(cp[:], t[:])
                    nc.sync.dma_start(out=dbg[name][:], in_=cp[:])

    nc.compile()
    return nc


def prep_inputs(inputs):
    """Host-side prep: per-core input dicts from the full problem inputs."""
    f32 = np.float32
    mmdt = _np_dt(MM_DT)
    msgdt = _np_dt(MSG_DT)
    actdt = _np_dt(ACT_DT)
    edge_src = np.asarray(inputs["edge_src"]).reshape(NG, NCL * KLIT)
    edge_dst = np.asarray(inputs["edge_dst"]).reshape(NG, NCL * KLIT)

    lmsg_w = np.asarray(inputs["lmsg_w"], f32)
    lmsg_b = np.asarray(inputs["lmsg_b"], f32)
    cmsg_w = np.asarray(inputs["cmsg_w"], f32)
    cmsg_b = np.asarray(inputs["cmsg_b"], f32)

    lmsgT_f = np.ascontiguousarray(np.transpose(lmsg_w, (2, 0, 1)))
    lmsgT = lmsgT_f.astype(actdt)
    cmsgT = np.ascontiguousarray(np.transpose(cmsg_w, (2, 0, 1))).astype(actdt)
    warmsrc = np.ascontiguousarray(lmsgT_f[:, 0:2, :]).astype(mmdt)
    lmsg_b01 = np.ascontiguousarray(lmsg_b[0:2].T)  # [128,2] f32
    cmsg_b01 = np.ascontiguousarray(cmsg_b[0:2].T)
    aggc_b = np.ascontiguousarray((KLIT * lmsg_b[2])[:, None])  # [128,1]

    def gate_pack(w):  # [512, din] -> [din, 4, 128] f32
        return np.ascontiguousarray(
            np.transpose(w.reshape(4, 128, -1), (2, 0, 1)))

    cu_wihT = gate_pack(np.asarray(inputs["cu_wih"], f32))
    cu_whhT = gate_pack(np.asarray(inputs["cu_whh"], f32))
    cu_pair = np.ascontiguousarray(
        np.stack([cu_wihT, cu_whhT], axis=2)).astype(actdt)  # [128,4,2,128]
    cu_b = np.ascontiguousarray(
        (np.asarray(inputs["cu_bih"], f32)
         + np.asarray(inputs["cu_bhh"], f32)).reshape(4, 128).T)
    lu_wih = np.asarray(inputs["lu_wih"], f32)  # [512, 256]
    lu_wihTa = gate_pack(lu_wih[:, :128]).astype(actdt)
    lu_wihTb = gate_pack(lu_wih[:, 128:])
    lu_whhT = gate_pack(np.asarray(inputs["lu_whh"], f32))
    # chunk 0: own h is cols [0:400] (k-tile 0), flip is [400:800] (k1)
    lu_pair0 = np.ascontiguousarray(
        np.stack([lu_whhT, lu_wihTb], axis=2)).astype(actdt)
    lu_pair1 = np.ascontiguousarray(
        np.stack([lu_wihTb, lu_whhT], axis=2)).astype(actdt)
    lu_b = np.ascontiguousarray(
        (np.asarray(inputs["lu_bih"], f32)
         + np.asarray(inputs["lu_bhh"], f32)).reshape(4, 128).T)

    vw0T = np.asarray(inputs["vote_w0"], f32).T.astype(mmdt)
    vw1T = np.asarray(inputs["vote_w1"], f32).T.astype(mmdt)
    vw2T = np.asarray(inputs["vote_w2"], f32).T.astype(mmdt)  # [128,1]
    vb = np.stack([np.asarray(inputs["vote_b0"], f32),
                   np.asarray(inputs["vote_b1"], f32)], axis=1)  # [128,2]

    h0l = (np.asarray(inputs["L_init_w"], f32)[:, 0]
           + np.asarray(inputs["L_init_b"], f32))  # [128]
    h0c = (np.asarray(inputs["C_init_w"], f32)[:, 0]
           + np.asarray(inputs["C_init_b"], f32))
    h0_lit = np.ascontiguousarray(
        np.broadcast_to(h0l[:, None], (128, NLIT))).astype(actdt)
    h0_cl = np.ascontiguousarray(
        np.broadcast_to(h0c[:, None], (128, NCL))).astype(actdt)

    cmsg_b2 = cmsg_b[2]  # [128]

    in_maps = []
    for g in range(NG):
        src = edge_src[g] - g * NNG          # local literal ids [0, 800)
        dst = edge_dst[g] - g * NNG - NLIT   # local clause ids [0, 1200)
        A = np.zeros((LCH * 128, NCL), f32)
        np.add.at(A, (src, dst), 1.0)
        deg = A.sum(axis=1)[:NLIT]           # literal degrees
        a_lc = np.ascontiguousarray(
            A.reshape(LCH, 128, NCL).transpose(1, 0, 2)).astype(msgdt)
        At = np.zeros((CCH * 128, NLIT), f32)
        At[:NCL] = A[:NLIT].T
        a_cl = np.ascontiguousarray(
            At.reshape(CCH, 128, NLIT).transpose(1, 0, 2)).astype(msgdt)
        aggl_b = np.ascontiguousarray(np.outer(cmsg_b2, deg))  # [128,800] f32

        in_maps.append(dict(
            a_lc=a_lc, a_cl=a_cl, h0_lit=h0_lit, h0_cl=h0_cl,
            lmsgT=lmsgT, lmsg_b=lmsg_b01, cmsgT=cmsgT, cmsg_b=cmsg_b01,
            aggc_b=aggc_b, aggl_b=aggl_b,
            cu_pair=cu_pair, cu_b=cu_b,
            lu_wihTa=lu_wihTa, lu_pair0=lu_pair0, lu_pair1=lu_pair1,
            lu_b=lu_b,
            vw0T=vw0T, vw1T=vw1T, vw2T=vw2T, vb=vb, warmsrc=warmsrc,
        ))
    return in_maps


_NC_CACHE = {}
LAST_RESULT = None


def kernel(**inputs):
    global LAST_RESULT
    key = "main"
    if key not in _NC_CACHE:
        _NC_CACHE[key] = build_nc()
    nc = _NC_CACHE[key]
    in_maps = prep_inputs(inputs)
    res = run_bass_kernel_spmd(nc, in_maps, list(range(NG)))
    LAST_RESULT = res
    vote_b2 = float(np.asarray(inputs["vote_b2"], np.float32)[0])
    n_vars = np.asarray(inputs["n_vars"]).astype(np.float32)
    sums = np.array([res.results[g]["out"][0, 0] for g in range(NG)],
                    np.float32)
    sums = sums + np.float32(NLIT * vote_b2)
    return (sums / (2.0 * n_vars)).astype(np.float32)
